# revision 1
# baseline (speedup 1.0000x reference)
"""Bilateral filter (joint/cross, 21-channel unaries, 3-channel guide) on 8 Trainium2 cores.

out[b,i,c,h,w] = sum_k wk * exp(-2*(I[b,i,p+dk]-I[b,i,p])^2) * Q[b,c,p+dk] / norm
(5x5 neighborhood minus center, zero padding, theta_alpha=1.5, theta_beta=0.5)

Sharding: pure data parallel over (batch, H-half) -> 8 shards, each core gets a
halo'd (132-row) padded shard and produces a (3,21,128,256) output block.

Per-core compute:
  - g[a][b-plane] = exp(-2*d^2 + ln(wk)) built with DVE subs + ACT Square/Exp.
  - 24 per-pixel products g (*) Q-window run on DVE + GpSimd (static split).
  - the 24-term accumulation runs on the PE array as identity-stationary
    float32r matmuls accumulating into PSUM (1 cycle/row at N=256).
  - normalization (reciprocal + multiply) reads PSUM directly; DMA out.
"""

import os
import sys

import numpy as np

_REPO = "/opt/trn_rl_repo"
if os.path.isdir(_REPO) and _REPO not in sys.path:
    sys.path.insert(0, _REPO)

import concourse.bacc as bacc
import concourse.bass as bass
import concourse.mybir as mybir
import concourse.tile as tile
from concourse.bass_utils import run_bass_kernel_spmd

F32 = mybir.dt.float32
F32R = mybir.dt.float32r

KS = 5
PAD = 2
THETA_ALPHA = 1.5
THETA_BETA = 0.5
EXP_SCALE = -1.0 / (2.0 * THETA_BETA * THETA_BETA)  # -2.0

B, CIN, NC_CH, H, W = 4, 3, 21, 256, 256
HOUT = 128           # output rows per core
HIN = HOUT + 2 * PAD  # 132 input rows per core (halo)
WP = W + 2 * PAD      # 260 padded cols
N_CORES = 8

# spatial (domain) weights wk[a][b], a/b in 0..4; center (2,2) excluded
_WK = np.exp(
    -(
        (np.arange(5)[:, None] - 2) ** 2 + (np.arange(5)[None, :] - 2) ** 2
    ).astype(np.float64)
    / (2.0 * THETA_ALPHA**2)
)
_LNWK = np.log(_WK)

# All 25 (a, b) plane indices in emission order; center stays (zeroed plane).
_PLANES = [(a, b) for a in range(5) for b in range(5)]


_BATCH_SUBS = True
_NORM_ON_ACT = False
_SUBS_ENG = "vector"  # "vector" | "gpsimd"
_G_DBUF = 3  # how many of the g tags get bufs=2
_PP_BUFS = 3
_PS_BUFS = 6
_EXACT_NORM = False  # norm via exact f32 DVE reduce (not f32r PE accumulate)
# Normalize tail: "vector" = DVE mult straight from PSUM;
# "act_gps" = ACT evacuates PSUM -> SBUF, GpSimd does the multiply
# (keeps the whole per-channel tail off the bottleneck DVE engine).
_NORM_MULT = "act_gps"


def _gps_split(ci, a):
    """Which product instructions run on GpSimd (vs DVE)."""
    return (a + ci) % 3 == 2


def _overlap_view(t, n_shift, width, elem_offset=0):
    """AP view [128, n_shift, width] of tile `t` where element (p, s, w)
    reads t[p, elem_offset + w + s] (both free strides 1 -> overlapping
    windows)."""
    ap = t[:] if not isinstance(t, bass.AP) else t
    pairs = [list(p) for p in ap.ap]
    part = pairs[0]
    return bass.AP(
        ap.tensor, ap.offset + elem_offset, [part, [1, n_shift], [1, width]]
    )


_PROG_CACHE = {}


def _build_program():
    """Build (once) the single-core Bass/Tile program shared by all 8 cores."""
    if "nc" in _PROG_CACHE:
        return _PROG_CACHE["nc"]

    nc = bacc.Bacc("TRN2", target_bir_lowering=False, debug=False)
    I_d = nc.dram_tensor("I_in", (CIN, HIN, WP), F32, kind="ExternalInput")
    Q_d = nc.dram_tensor("Q_in", (NC_CH, HIN, WP), F32, kind="ExternalInput")
    E_d = nc.dram_tensor("EYE", (128, 128), F32, kind="ExternalInput")
    O_d = nc.dram_tensor("OUT", (CIN, NC_CH, HOUT, W), F32, kind="ExternalOutput")

    with tile.TileContext(nc) as tc:
        with (
            tc.tile_pool(name="qp", bufs=1) as qp,
            tc.tile_pool(name="ip", bufs=1 if _EXACT_NORM else 2) as ip,
            tc.tile_pool(name="gp", bufs=1) as gp,
            tc.tile_pool(name="gp2", bufs=2) as gp2,
            tc.tile_pool(name="pp", bufs=_PP_BUFS) as pp,
            tc.tile_pool(
                name="op", bufs=3 if _NORM_MULT == "act_gps" else 6
            ) as op,
            tc.tile_pool(name="cp", bufs=1) as cp,
            tc.tile_pool(name="rp", bufs=2) as rp,
            tc.tile_pool(name="rp1", bufs=1) as rp1,
            tc.tile_pool(name="ps", bufs=_PS_BUFS, space="PSUM") as ps,
        ):
            eye = cp.tile([128, 128], F32, tag="eye")
            nc.sync.dma_start(eye[:], E_d[:, :])
            # PE f32r matmuls require operands produced as float32r; the
            # identity's 0/1 values are exact under any rounding. wk is
            # separable (wr[a]*wc[b]): wr rides the Exp bias, wc rides
            # scaled-identity stationaries (3 distinct values; wc[2]=1).
            eye_r = cp.tile([128, 128], F32R, tag="eye_r")
            nc.vector.tensor_copy(eye_r[:], eye[:])
            _wc = np.exp(-((np.arange(5) - 2.0) ** 2) / (2.0 * THETA_ALPHA**2))
            eye_b = {}
            for b in range(5):
                if b == 2:
                    eye_b[b] = eye_r
                elif (4 - b) in eye_b and abs(_wc[b] - _wc[4 - b]) < 1e-12:
                    eye_b[b] = eye_b[4 - b]
                else:
                    t = cp.tile([128, 128], F32R, tag=f"eye_b{b}")
                    nc.vector.tensor_scalar_mul(t[:], eye[:], float(_wc[b]))
                    eye_b[b] = t

            # per-partition bias tiles holding ln(wr[a]) for the Exp stage
            bias_t = {}
            for a in range(5):
                t = cp.tile([128, 1], F32, tag=f"bias{a}")
                nc.gpsimd.memset(t[:], float(np.log(_wc[a])))
                bias_t[a] = t

            # Q row-windows, one c-batched tile per vertical offset a:
            # qa[a] is [128, 21, 260] holding rows a..a+127 of every channel.
            qa = {}
            for a in range(5):
                t = qp.tile([128, NC_CH, WP], F32, tag=f"qa{a}")
                nc.sync.dma_start(t[:], Q_d[:, a : a + 128, :].transpose([1, 0, 2]))
                qa[a] = t

            for i in range(CIN):
                i0 = ip.tile([128, WP], F32, tag="i0")
                nc.sync.dma_start(i0[:], I_d[i, 2:130, :])
                ia = {}
                for a in range(5):
                    if a == 2:
                        ia[a] = i0
                        continue
                    t = ip.tile([128, WP], F32, tag=f"ia{a}")
                    nc.sync.dma_start(t[:], I_d[i, a : a + 128, :])
                    ia[a] = t

                # g[a]: [128, 5(b), 256] appearance*domain weights
                g = {}
                sub_eng = nc.gpsimd if _SUBS_ENG == "gpsimd" else nc.vector
                for a in range(5):
                    pool = gp2 if a < _G_DBUF else gp
                    gb = pool.tile([128, 5, W], F32, tag=f"g{a}")
                    if _BATCH_SUBS:
                        iav = _overlap_view(ia[a], 5, W)
                        i0b = i0[:, None, 2 : 2 + W].broadcast_to([128, 5, W])
                        sub_eng.tensor_sub(gb[:], iav, i0b)
                    else:
                        for b in range(5):
                            sub_eng.tensor_sub(
                                gb[:, b, :], ia[a][:, b : b + W], i0[:, 2 : 2 + W]
                            )
                    nc.scalar.activation(
                        gb[:], gb[:], mybir.ActivationFunctionType.Square
                    )
                    nc.scalar.activation(
                        gb[:],
                        gb[:],
                        mybir.ActivationFunctionType.Exp,
                        bias=bias_t[a][:],
                        scale=EXP_SCALE,
                    )
                    g[a] = gb
                # kill the (excluded) center tap
                nc.gpsimd.memset(g[2][:, 2, :], 0.0)

                # ---- norm (ci == -1) and the 21 unary channels.
                # Per (a): products g (*) Q-window -> f32r planes; PE
                # identity-matmuls accumulate the 25 planes into PSUM.
                # The per-channel tail (recip / normalize+store) is emitted one
                # iteration LATE so it never head-of-line blocks the DVE queue
                # behind the PE accumulation group it depends on.
                recip = None
                pending = None  # (ci, acc) awaiting its tail ops

                def _flush_tail():
                    nonlocal recip, pending
                    if pending is None:
                        return
                    pci, pacc = pending
                    pending = None
                    if pci < 0:
                        recip = rp.tile([128, W], F32, tag="recip")
                        rscratch = rp1.tile([128, W], F32, tag="rscratch")
                        nc.vector.reciprocal_approx_accurate(
                            recip[:], pacc[:], rscratch[:]
                        )
                    else:
                        ob = op.tile([128, W], F32, tag="ob")
                        if _NORM_MULT == "act_gps":
                            ob1 = op.tile([128, W], F32, tag="ob1")
                            nc.scalar.copy(ob1[:], pacc[:])
                            nc.gpsimd.tensor_mul(ob[:], ob1[:], recip[:])
                        else:
                            nc.vector.tensor_mul(ob[:], pacc[:], recip[:])
                        nc.sync.dma_start(O_d[i, pci, :, :], ob[:])

                if _EXACT_NORM:
                    # norm = sum over all 25 (a,b) planes of g, exact f32:
                    # per-a reduce over b (strided view), then chain-add.
                    ra = rp.tile([128, W], F32, tag="nra")
                    rb = rp.tile([128, W], F32, tag="nrb")
                    for a in range(5):
                        gap = g[a][:]
                        part = [list(p) for p in gap.ap][0]
                        bview = bass.AP(
                            gap.tensor, gap.offset, [part, [1, W], [W, 5]]
                        )
                        dst = ra if a == 0 else rb
                        nc.vector.tensor_reduce(
                            dst[:], bview, axis=mybir.AxisListType.X,
                            op=mybir.AluOpType.add,
                        )
                        if a > 0:
                            nc.vector.tensor_add(ra[:], ra[:], rb[:])
                    recip = rp.tile([128, W], F32, tag="recip")
                    rscratch = rp.tile([128, W], F32, tag="rscratch")
                    nc.vector.reciprocal_approx_accurate(
                        recip[:], ra[:], rscratch[:]
                    )

                ci_list = (
                    list(range(NC_CH)) if _EXACT_NORM
                    else [-1] + list(range(NC_CH))
                )
                for ci in ci_list:
                    acc = ps.tile([128, W], F32, tag="acc")
                    idx = 0
                    for a in range(5):
                        use_gps = ci >= 0 and _gps_split(ci, a)
                        pb = pp.tile(
                            [128, 5, W], F32R, tag="pbg" if use_gps else "pbv"
                        )
                        if ci < 0:
                            # norm channel: planes are just g itself
                            # (f32r-rounding copy).
                            if _NORM_ON_ACT:
                                nc.scalar.copy(pb[:], g[a][:])
                            else:
                                nc.vector.tensor_copy(pb[:], g[a][:])
                        else:
                            qv = _overlap_view(qa[a], 5, W, elem_offset=ci * WP)
                            eng = nc.gpsimd if use_gps else nc.vector
                            eng.tensor_mul(pb[:], g[a][:], qv)
                        if a == 2:
                            _flush_tail()
                        for b in range(5):
                            if a == 2 and b == 2:
                                continue  # center plane is identically zero
                            nc.tensor.matmul(
                                acc[:],
                                eye_b[b][:],
                                pb[:, b, :],
                                start=(idx == 0),
                                stop=(idx == 23),
                            )
                            idx += 1
                    pending = (ci, acc)
                _flush_tail()

    nc.compile()
    _PROG_CACHE["nc"] = nc
    return nc


def _make_in_maps(Q, I):
    Q = np.ascontiguousarray(np.asarray(Q, dtype=np.float32))
    I = np.ascontiguousarray(np.asarray(I, dtype=np.float32))
    Ip = np.zeros((B, CIN, H + 2 * PAD, WP), np.float32)
    Ip[:, :, PAD : PAD + H, PAD : PAD + W] = I
    Qp = np.zeros((B, NC_CH, H + 2 * PAD, WP), np.float32)
    Qp[:, :, PAD : PAD + H, PAD : PAD + W] = Q
    eye = np.ascontiguousarray(np.eye(128, dtype=np.float32))
    in_maps = []
    for core in range(N_CORES):
        b, half = divmod(core, 2)
        h0 = half * HOUT
        in_maps.append(
            {
                "I_in": np.ascontiguousarray(Ip[b, :, h0 : h0 + HIN, :]),
                "Q_in": np.ascontiguousarray(Qp[b, :, h0 : h0 + HIN, :]),
                "EYE": eye,
            }
        )
    return in_maps


def _assemble(results):
    out = np.zeros((B, CIN, NC_CH, H, W), np.float32)
    for core in range(N_CORES):
        b, half = divmod(core, 2)
        h0 = half * HOUT
        out[b, :, :, h0 : h0 + HOUT, :] = results[core]["OUT"]
    return out


def kernel(Q: np.ndarray, I: np.ndarray) -> np.ndarray:
    nc = _build_program()
    in_maps = _make_in_maps(Q, I)
    res = run_bass_kernel_spmd(nc, in_maps, core_ids=list(range(N_CORES)))
    return _assemble(res.results)



# revision 2
# speedup vs baseline: 1.5284x; 1.5284x over previous
"""Bilateral filter (joint/cross, 21-channel unaries, 3-channel guide) on 8 Trainium2 cores.

out[b,i,c,h,w] = sum_k wk * exp(-2*(I[b,i,p+dk]-I[b,i,p])^2) * Q[b,c,p+dk] / norm
(5x5 neighborhood minus center, zero padding, theta_alpha=1.5, theta_beta=0.5)

Sharding: pure data parallel over (batch, H-half) -> 8 shards, each core gets a
halo'd (132-row) padded shard and produces a (3,21,128,256) output block.

Per-core compute (fp16 datapath; tolerance is 2e-2 so fp16 has ~10x margin):
  - host ships I/Q pre-padded + fp16, plus fp16 identity stationaries
    pre-scaled by the separable column weights wc (eye, wc1*eye, wc2*eye).
  - g[a] = exp(-2*d^2 + ln(wr[a])) built with one DVE fp16 sub (2x mode) +
    ACT Square/Exp per vertical offset a; row weights wr ride the Exp bias.
  - per (i,c,a): 5-plane fp16 product g (*) Q-window on DVE (2x) or GpSimd
    (static split tuned against the cost model).
  - 24-term neighbor accumulation: fp16 identity-stationary matmuls into
    f32 PSUM (107ns per plane at full PE clock).
  - norm channel matmuls fp16 g planes directly (no copies); recip on DVE;
    per-channel tail = ACT evac (PSUM->SBUF) + GpSimd multiply -> f32 out.
"""

import os
import sys

import numpy as np

_REPO = "/opt/trn_rl_repo"
if os.path.isdir(_REPO) and _REPO not in sys.path:
    sys.path.insert(0, _REPO)

import concourse.bacc as bacc
import concourse.bass as bass
import concourse.mybir as mybir
import concourse.tile as tile
from concourse.bass_utils import run_bass_kernel_spmd

F32 = mybir.dt.float32
F16 = mybir.dt.float16

KS = 5
PAD = 2
THETA_ALPHA = 1.5
THETA_BETA = 0.5
EXP_SCALE = -1.0 / (2.0 * THETA_BETA * THETA_BETA)  # -2.0

B, CIN, NC_CH, H, W = 4, 3, 21, 256, 256
HOUT = 128           # output rows per core
HIN = HOUT + 2 * PAD  # 132 input rows per core (halo)
WP = W + 2 * PAD      # 260 padded cols
N_CORES = 8

# separable spatial weights: wk(a,b) = wr[a]*wc[b], center (2,2) excluded
_WC = np.exp(-((np.arange(5) - 2.0) ** 2) / (2.0 * THETA_ALPHA**2))


def _prod_on_gps(i, ci, a):
    """Which product groups run on GpSimd (vs DVE). ~19% of 315 groups."""
    return (a + 5 * ci + 2 * i) % 5 == 2


def _overlap_view(t, n_shift, width, elem_offset=0):
    """AP view [128, n_shift, width] of tile `t` where element (p, s, w)
    reads t[p, elem_offset + w + s] (both free strides 1 -> overlapping
    windows)."""
    ap = t[:] if not isinstance(t, bass.AP) else t
    pairs = [list(p) for p in ap.ap]
    part = pairs[0]
    return bass.AP(
        ap.tensor, ap.offset + elem_offset, [part, [1, n_shift], [1, width]]
    )


_PROG_CACHE = {}


def _build_program():
    """Build (once) the single-core Bass/Tile program shared by all 8 cores."""
    if "nc" in _PROG_CACHE:
        return _PROG_CACHE["nc"]

    nc = bacc.Bacc("TRN2", target_bir_lowering=False, debug=False)
    I_d = nc.dram_tensor("I_in", (CIN, HIN, WP), F16, kind="ExternalInput")
    Q_d = nc.dram_tensor("Q_in", (NC_CH, HIN, WP), F16, kind="ExternalInput")
    E_d = nc.dram_tensor("EYE3", (3, 128, 128), F16, kind="ExternalInput")
    O_d = nc.dram_tensor("OUT", (CIN, NC_CH, HOUT, W), F32, kind="ExternalOutput")

    with tile.TileContext(nc) as tc:
        with (
            tc.tile_pool(name="qp", bufs=1) as qp,
            tc.tile_pool(name="ip", bufs=2) as ip,
            tc.tile_pool(name="gp", bufs=2) as gp,
            tc.tile_pool(name="pp", bufs=4) as pp,
            tc.tile_pool(name="ppg", bufs=3) as ppg,
            tc.tile_pool(name="op", bufs=4) as op,
            tc.tile_pool(name="ep", bufs=4) as ep,
            tc.tile_pool(name="cp", bufs=1) as cp,
            tc.tile_pool(name="rp", bufs=2) as rp,
            tc.tile_pool(name="rp1", bufs=1) as rp1,
            tc.tile_pool(name="ps", bufs=6, space="PSUM") as ps,
        ):
            # fp16 identity stationaries pre-scaled by wc (host-provided):
            # slice j of EYE3 = eye * wc_level[j], levels [1, wc1, wc2].
            eye_t = cp.tile([128, 3, 128], F16, tag="eye3")
            nc.sync.dma_start(eye_t[:], E_d[:, :, :].transpose([1, 0, 2]))
            # eye_b[b]: stationary for column offset b (wc symmetric)
            _lvl = [2, 1, 0, 1, 2]
            eye_b = [eye_t[:, _lvl[b], :] for b in range(5)]

            # per-partition bias tiles holding ln(wr[a]) for the Exp stage
            bias_t = {}
            for a in range(5):
                if (4 - a) in bias_t:
                    bias_t[a] = bias_t[4 - a]
                    continue
                t = cp.tile([128, 1], F32, tag=f"bias{a}")
                nc.gpsimd.memset(t[:], float(np.log(_WC[a])))
                bias_t[a] = t

            # Q row-windows, one c-batched tile per vertical offset a:
            # qa[a] is [128, 21, 260] fp16 holding rows a..a+127 of every channel.
            qa = {}
            for a in range(5):
                t = qp.tile([128, NC_CH, WP], F16, tag=f"qa{a}")
                nc.sync.dma_start(t[:], Q_d[:, a : a + 128, :].transpose([1, 0, 2]))
                qa[a] = t

            for i in range(CIN):
                i0 = ip.tile([128, WP], F16, tag="i0")
                nc.sync.dma_start(i0[:], I_d[i, 2:130, :])
                ia = {}
                for a in range(5):
                    if a == 2:
                        ia[a] = i0
                        continue
                    t = ip.tile([128, WP], F16, tag=f"ia{a}")
                    nc.sync.dma_start(t[:], I_d[i, a : a + 128, :])
                    ia[a] = t

                # g[a]: [128, 5(b), 256] fp16 appearance*domain weights
                g = {}
                for a in range(5):
                    gb = gp.tile([128, 5, W], F16, tag=f"g{a}")
                    iav = _overlap_view(ia[a], 5, W)
                    i0b = i0[:, None, 2 : 2 + W].broadcast_to([128, 5, W])
                    nc.vector.tensor_sub(gb[:], iav, i0b)
                    nc.scalar.activation(
                        gb[:], gb[:], mybir.ActivationFunctionType.Square
                    )
                    nc.scalar.activation(
                        gb[:],
                        gb[:],
                        mybir.ActivationFunctionType.Exp,
                        bias=bias_t[a][:],
                        scale=EXP_SCALE,
                    )
                    g[a] = gb
                # kill the (excluded) center tap
                nc.gpsimd.memset(g[2][:, 2, :], 0.0)

                # ---- norm (ci == -1) and the 21 unary channels.
                # Per (a): products g (*) Q-window -> fp16 planes; PE
                # identity-matmuls accumulate the 24 planes into f32 PSUM.
                # The per-channel tail (recip / evac+normalize+store) is
                # emitted one iteration LATE so it never head-of-line blocks
                # its engine queue behind the PE group it depends on.
                recip = None
                pending = None  # (ci, acc) awaiting its tail ops

                def _flush_tail():
                    nonlocal recip, pending
                    if pending is None:
                        return
                    pci, pacc = pending
                    pending = None
                    if pci < 0:
                        recip = rp.tile([128, W], F32, tag="recip")
                        rscratch = rp1.tile([128, W], F32, tag="rscratch")
                        nc.vector.reciprocal_approx_accurate(
                            recip[:], pacc[:], rscratch[:]
                        )
                    else:
                        # ACT evacuates PSUM -> SBUF f32; GpSimd multiplies by
                        # the reciprocal (tail stays off the loaded DVE).
                        ob1 = ep.tile([128, W], F32, tag="ob1")
                        nc.scalar.copy(ob1[:], pacc[:])
                        ob = op.tile([128, W], F32, tag="ob")
                        nc.gpsimd.tensor_mul(ob[:], ob1[:], recip[:])
                        nc.sync.dma_start(O_d[i, pci, :, :], ob[:])

                for ci in [-1] + list(range(NC_CH)):
                    acc = ps.tile([128, W], F32, tag="acc")
                    idx = 0
                    for a in range(5):
                        if ci < 0:
                            pb = g[a]  # norm: accumulate g planes directly
                        else:
                            use_gps = _prod_on_gps(i, ci, a)
                            pool = ppg if use_gps else pp
                            pb = pool.tile(
                                [128, 5, W], F16, tag="pbg" if use_gps else "pbv"
                            )
                            qv = _overlap_view(qa[a], 5, W, elem_offset=ci * WP)
                            eng = nc.gpsimd if use_gps else nc.vector
                            eng.tensor_mul(pb[:], g[a][:], qv)
                        if a == 2:
                            _flush_tail()
                        for b in range(5):
                            if a == 2 and b == 2:
                                continue  # center plane is identically zero
                            nc.tensor.matmul(
                                acc[:],
                                eye_b[b],
                                pb[:, b, :],
                                start=(idx == 0),
                                stop=(idx == 23),
                            )
                            idx += 1
                    pending = (ci, acc)
                _flush_tail()

    nc.compile()
    _PROG_CACHE["nc"] = nc
    return nc


def _make_in_maps(Q, I):
    Q = np.asarray(Q, dtype=np.float32)
    I = np.asarray(I, dtype=np.float32)
    Ip = np.zeros((B, CIN, H + 2 * PAD, WP), np.float16)
    Ip[:, :, PAD : PAD + H, PAD : PAD + W] = I.astype(np.float16)
    Qp = np.zeros((B, NC_CH, H + 2 * PAD, WP), np.float16)
    Qp[:, :, PAD : PAD + H, PAD : PAD + W] = Q.astype(np.float16)
    eye = np.eye(128, dtype=np.float16)
    eye3 = np.ascontiguousarray(
        np.stack([eye, eye * np.float16(_WC[1]), eye * np.float16(_WC[0])])
    )
    in_maps = []
    for core in range(N_CORES):
        b, half = divmod(core, 2)
        h0 = half * HOUT
        in_maps.append(
            {
                "I_in": np.ascontiguousarray(Ip[b, :, h0 : h0 + HIN, :]),
                "Q_in": np.ascontiguousarray(Qp[b, :, h0 : h0 + HIN, :]),
                "EYE3": eye3,
            }
        )
    return in_maps


def _assemble(results):
    out = np.zeros((B, CIN, NC_CH, H, W), np.float32)
    for core in range(N_CORES):
        b, half = divmod(core, 2)
        h0 = half * HOUT
        out[b, :, :, h0 : h0 + HOUT, :] = results[core]["OUT"]
    return out


def kernel(Q: np.ndarray, I: np.ndarray) -> np.ndarray:
    nc = _build_program()
    in_maps = _make_in_maps(Q, I)
    res = run_bass_kernel_spmd(nc, in_maps, core_ids=list(range(N_CORES)))
    return _assemble(res.results)


# revision 53
# speedup vs baseline: 1.9861x; 1.2995x over previous
"""Bilateral filter (joint/cross, 21-channel unaries, 3-channel guide) on 8 Trainium2 cores.

out[b,i,c,h,w] = sum_k wk * exp(-2*(I[b,i,p+dk]-I[b,i,p])^2) * Q[b,c,p+dk] / norm
(5x5 neighborhood minus center, zero padding, theta_alpha=1.5, theta_beta=0.5)

Sharding: pure data parallel over (batch, H-half) -> 8 shards, each core gets a
halo'd (132-row) padded shard and produces a (3,21,128,256) output block.

Per-core compute (fp16 datapath; tolerance is 2e-2 so fp16 has ~30x margin):
  - host ships I/Q pre-padded fp16, ln(Q+8) fp16, and fp16 identity
    stationaries pre-scaled by the column weights wc. Q lands in one SBUF
    mega-tile [128, 5(a), 21(c), 260] of row-shifted copies so a whole
    channel's vertical-offset groups multiply in ONE DVE instruction.
  - g built per vertical offset a: one DVE fp16 sub (2x mode) writes a d
    slice of the g mega-tile [128, 25, 256]; ACT Square keeps d^2 in a
    separate mega-tile, ACT Exp emits g; row weights wr ride the Exp bias,
    column weights wc ride the matmul stationaries. The next guide's
    g-build is emitted mid-way through the previous one's channels so the
    ACT chain hides behind product work.
  - per channel: one DVE fp16 tensor_mul (2x_1p mode, ~2.7us) covers 4 of
    the 5 a-groups; one group is donated to GpSimd (emitted with lookahead).
    Two channels are instead routed entirely through PE+ACT: two identity
    matmuls build arg = -2*d^2 + ln(Q+8) in PSUM per plane and a single
    ACT Exp (bias ln wr) emits the product plane g*(Q+8); the tail's
    *recip - 8 removes the shift exactly. This moves multiply work onto
    otherwise-idle PE/ACT cycles, balancing DVE/Pool/PE at ~185us each.
  - 24-term neighbor accumulation: fp16 identity-stationary matmuls into
    f32 PSUM (107ns per plane at full PE clock), 6 PSUM banks rotating.
  - norm channel matmuls the fp16 g planes directly; recip on DVE; tails
    are ACT-evac + GpSimd multiply (DVE-direct near the program end).
"""

import os
import sys

import numpy as np

_REPO = "/opt/trn_rl_repo"
if os.path.isdir(_REPO) and _REPO not in sys.path:
    sys.path.insert(0, _REPO)

import concourse.bacc as bacc
import concourse.bass as bass
import concourse.mybir as mybir
import concourse.tile as tile
from concourse.bass_utils import run_bass_kernel_spmd

F32 = mybir.dt.float32
F16 = mybir.dt.float16

KS = 5
PAD = 2
THETA_ALPHA = 1.5
THETA_BETA = 0.5
EXP_SCALE = -1.0 / (2.0 * THETA_BETA * THETA_BETA)  # -2.0

B, CIN, NC_CH, H, W = 4, 3, 21, 256, 256
HOUT = 128           # output rows per core
HIN = HOUT + 2 * PAD  # 132 input rows per core (halo)
WP = W + 2 * PAD      # 260 padded cols
N_CORES = 8
QSTRIDE = NC_CH * WP  # elems per a-slice of the Q mega-tile

# separable spatial weights: wk(a,b) = wr[a]*wc[b], center (2,2) excluded
_WC = np.exp(-((np.arange(5) - 2.0) ** 2) / (2.0 * THETA_ALPHA**2))

CFG = {
    "donate_skip_mod": 8,   # channel donates a group unless (3ci+i)%mod==0
    "donate2_mod": 0,       # every m2-th channel donates both end groups
    "tail_dve_mod": 0,      # 0: all tails ACT+GpS; k: every k-th on DVE
    "lookahead": 2,         # GpS donated-product emission lookahead
    "gbuild_at": 10,        # emit next i's g-build at this channel index
    "drain_dve": 1,         # last channels of last i run DVE-only
    "early_split": 3,       # first channels of i=0 use per-group products
    "late_split": 2,        # last channels of last i use per-group products
    "qm_chunk0": 3,         # channels in the first Q-mega DMA chunk
    "route": ((1, 6), (2, 9)),  # channels whose products run as PE+ACT exp
    "route_group": False,    # arg PSUM at a-group granularity (else per-plane)
    "route_defer": 2,
    "route2": (),
    "pair_prod": 0,
    "d2p_bufs": 2,
    "pp2_bufs": 1,
    "ep_bufs": 4,
    "op_bufs": 4,       # emit routed sum-matmuls this many channels late
    "pp_bufs": 3,
    "ppg_bufs": 4,
    "ps_bufs": 6,
    "gp_bufs": 2,
}


def _donate_a(i, ci):
    """Which a-groups of channel (i, ci) run on GpSimd (list of 0/2 ends)."""
    if (i, ci) in CFG["route"] or (i, ci) in CFG["route2"]:
        return []  # routed channels build their products on PE+ACT
    if i == CIN - 1 and ci >= NC_CH - CFG["drain_dve"]:
        return []  # keep the Pool queue short near program end
    if i == 0 and ci < CFG["early_split"]:
        return [0]  # a=0 builds first: Pool starts as soon as Exp(0) lands
    if CFG["donate_skip_mod"] and (3 * ci + i) % CFG["donate_skip_mod"] == 0:
        return []
    m2 = CFG["donate2_mod"]
    if m2 and (3 * ci + i) % m2 == m2 - 1:
        return [0, 4]
    return [0] if (ci + i) % 2 == 0 else [4]


def _tail_on_dve(i, ci):
    if i == CIN - 1 and ci >= NC_CH - CFG["drain_dve"]:
        return True
    m = CFG["tail_dve_mod"]
    return bool(m) and (3 * ci + i) % m == 0


def _view(t, dims, elem_offset=0):
    """AP view of tile `t` with explicit free dims [[stride, n], ...]."""
    ap = t[:] if not isinstance(t, bass.AP) else t
    part = [list(p) for p in ap.ap][0]
    return bass.AP(ap.tensor, ap.offset + elem_offset, [part] + dims)


_PROG_CACHE = {}


def _build_program():
    """Build (once) the single-core Bass/Tile program shared by all 8 cores."""
    if "nc" in _PROG_CACHE:
        return _PROG_CACHE["nc"]

    nc = bacc.Bacc("TRN2", target_bir_lowering=False, debug=False)
    I_d = nc.dram_tensor("I_in", (CIN, HIN, WP), F16, kind="ExternalInput")
    Q_d = nc.dram_tensor("Q_in", (NC_CH, HIN, WP), F16, kind="ExternalInput")
    E_d = nc.dram_tensor("EYE5", (5, 128, 128), F16, kind="ExternalInput")
    L_d = nc.dram_tensor("LQ_in", (NC_CH, HIN, WP), F16, kind="ExternalInput")
    O_d = nc.dram_tensor("OUT", (CIN, NC_CH, HOUT, W), F32, kind="ExternalOutput")

    with tile.TileContext(nc) as tc:
        with (
            tc.tile_pool(name="qp", bufs=1) as qp,
            tc.tile_pool(name="ip", bufs=2) as ip,
            tc.tile_pool(name="gp", bufs=CFG["gp_bufs"]) as gp,
            tc.tile_pool(name="pp", bufs=CFG["pp_bufs"]) as pp,
            tc.tile_pool(name="ppg", bufs=CFG["ppg_bufs"]) as ppg,
            tc.tile_pool(name="op", bufs=CFG["op_bufs"]) as op,
            tc.tile_pool(name="ep", bufs=CFG["ep_bufs"]) as ep,
            tc.tile_pool(name="cp", bufs=1) as cp,
            tc.tile_pool(name="rp", bufs=2) as rp,
            tc.tile_pool(name="rp1", bufs=1) as rp1,
            tc.tile_pool(name="d2p", bufs=CFG["d2p_bufs"]) as d2p,
            tc.tile_pool(name="lqp", bufs=2) as lqp,
            tc.tile_pool(name="ppe", bufs=2) as ppe,
            tc.tile_pool(name="ps", bufs=CFG["ps_bufs"], space="PSUM") as ps,
            tc.tile_pool(name="psa", bufs=2, space="PSUM") as psa,
        ):
            # fp16 identity stationaries pre-scaled by wc (host-provided):
            # slice j of EYE5 = eye * [1, wc1, wc2, -2, -8][j].
            eye_t = cp.tile([128, 5, 128], F16, tag="eye5")
            _lvl = [2, 1, 0, 1, 2]
            eye_b = [eye_t[:, _lvl[b], :] for b in range(5)]
            eye_n2 = eye_t[:, 3, :]
            eye_n8 = eye_t[:, 4, :]

            # per-partition bias tiles holding ln(wr[a]) for the Exp stage
            bias_t = {}
            for a in range(5):
                if (4 - a) in bias_t:
                    bias_t[a] = bias_t[4 - a]
                    continue
                t = cp.tile([128, 1], F32, tag=f"bias{a}")
                nc.gpsimd.memset(t[:], float(np.log(_WC[a])))
                bias_t[a] = t

            # Q mega-tile: [128, 5(a), 21(c), 260] fp16, a-slice holds rows
            # a..a+127 of every channel (5 row-shifted copies of Q). DMAs are
            # emitted AFTER the first g-build (channel chunks) so the first
            # guide's subs/Square/Exp/norm don't queue behind ~19us of Q
            # transfer on the DMA device.
            qm = qp.tile([128, 5, NC_CH, WP], F16, tag="qm")

            def _emit_qm_dmas():
                bounds = [0, CFG["qm_chunk0"]]
                while bounds[-1] < NC_CH:
                    bounds.append(min(NC_CH, bounds[-1] + 6))
                for ck, (c0, c1) in enumerate(zip(bounds[:-1], bounds[1:])):
                    for a in range(5):
                        nc.sync.dma_start(
                            qm[:, a, c0:c1, :],
                            Q_d[c0:c1, a : a + 128, :].transpose([1, 0, 2]),
                        )
                    if ck == 0:
                        # eye load rides the idle DVE sequencer so it doesn't
                        # delay the critical im/chunk0 issues on SP
                        nc.vector.dma_start(
                            eye_t[:], E_d[:, :, :].transpose([1, 0, 2])
                        )

            def qwin(ci, alo, na):
                """Q-window view [128, na(a), 5(b), 256]: (p,a,b,w) reads
                Q[ci, p + alo + a, w + b] (padded coords)."""
                return _view(
                    qm,
                    [[QSTRIDE, na], [1, 5], [1, W]],
                    elem_offset=alo * QSTRIDE + ci * WP,
                )

            # g mega-tile per guide channel [128, 25, 256]: plane 5a+b holds
            # the fp16 appearance*row weight for offset (a-2, b-2). Built
            # EARLY (mid-way through the previous channel's unaries) so the
            # ACT Square/Exp chain overlaps the previous i's product work.
            gm_tiles = {}
            gb_state = {}  # gi -> (i0, ia, gm, next_a)

            def _emit_gbuild_step(gi, n_groups=5):
                """Emit DMAs (first call) and up to n_groups a-group
                sub/Square/Exp chains of guide gi's g-build. Spreading the
                calls keeps the ACT queue from starving the PSUM evacs."""
                if gi >= CIN or gm_tiles.get(gi) is not None:
                    return
                if gi not in gb_state:
                    # one DMA for all 5 row-shifted copies: im[p,a,w] =
                    # I[gi, p+a, w] (overlapping-row source AP)
                    im = ip.tile([128, 5, WP], F16, tag="im")
                    iap = I_d[gi, 0:128, :]
                    src = bass.AP(
                        iap.tensor, iap.offset, [[WP, 128], [WP, 5], [1, WP]]
                    )
                    nc.sync.dma_start(im[:], src)
                    gm = gp.tile([128, 25, W], F16, tag="gm")
                    d2m = d2p.tile([128, 25, W], F16, tag="d2m")
                    gb_state[gi] = [im, gm, d2m, 0]
                im, gm, d2m, a0 = gb_state[gi]
                for a in range(a0, min(5, a0 + n_groups)):
                    gsl = gm[:, 5 * a : 5 * a + 5, :]
                    iav = _view(im, [[1, 5], [1, W]], elem_offset=a * WP)
                    i0b = _view(im, [[0, 5], [1, W]], elem_offset=2 * WP + 2)
                    nc.vector.tensor_sub(gsl, iav, i0b)
                    d2sl = d2m[:, 5 * a : 5 * a + 5, :]
                    if gi == 0 and a < CFG["dve_square"]:
                        nc.vector.tensor_mul(d2sl, gsl, gsl)
                    else:
                        nc.scalar.activation(
                            d2sl, gsl, mybir.ActivationFunctionType.Square
                        )
                    nc.scalar.activation(
                        gsl,
                        d2sl,
                        mybir.ActivationFunctionType.Exp,
                        bias=bias_t[a][:],
                        scale=EXP_SCALE,
                    )
                gb_state[gi][2] = min(5, a0 + n_groups)
                if gb_state[gi][2] == 5:
                    # kill the (excluded) center tap (ACT: stays ordered
                    # after Exp(a=2) without blocking the Pool queue)
                    nc.scalar.memzero(gm[:, 12, :])
                    gm_tiles[gi] = gm

            _emit_gbuild_step(0)
            _emit_qm_dmas()
            for i in range(CIN):
                gm, d2m = gm_tiles[i]

                # ---- norm (ci == -1) and the 21 unary channels.
                # DVE multiplies 4 a-groups per channel in one instruction;
                # the donated group runs on GpSimd (emitted with lookahead).
                # The per-channel tail is emitted one iteration late so it
                # never head-of-line blocks its engine queue.
                recip = None
                pendings = []  # (ci, acc, routed) awaiting tail ops
                deferred = {}  # position -> (routed_ci, planes)
                gps_emitted = {}  # ci -> (a_d, pbg) | None

                lq_tiles = {}

                def _ensure_gps(cj):
                    if cj in gps_emitted or cj >= NC_CH:
                        return
                    if (i, cj) in CFG["route"] or (i, cj) in CFG["route2"]:
                        # prefetch ln(Q+8) row-shifted copies (host-computed)
                        lq = lqp.tile([128, 5, WP], F16, tag="lq")
                        lap = L_d[cj, 0:128, :]
                        nc.sync.dma_start(
                            lq[:],
                            bass.AP(
                                lap.tensor, lap.offset,
                                [[WP, 128], [WP, 5], [1, WP]],
                            ),
                        )
                        lq_tiles[cj] = lq
                        gps_emitted[cj] = []
                        return
                    dons = []
                    for a_d in donate_ovr.get(cj, _donate_a(i, cj)):
                        pbg = ppg.tile([128, 5, W], F16, tag="pbg")
                        nc.gpsimd.tensor_mul(
                            pbg[:],
                            gm[:, 5 * a_d : 5 * a_d + 5, :],
                            qwin(cj, a_d, 1),
                        )
                        dons.append((a_d, pbg))
                    gps_emitted[cj] = dons

                def _flush_tail():
                    nonlocal recip
                    if not pendings:
                        return
                    pci, pacc, prouted = pendings.pop(0)
                    if pci < 0:
                        # ~18-bit reciprocal: plenty against the 2e-2 gate
                        recip = rp.tile([128, W], F32, tag="recip")
                        nc.vector.reciprocal_approx_fast(recip[:], pacc[:])
                    elif _tail_on_dve(i, pci):
                        ob = op.tile([128, W], F32, tag="ob")
                        nc.vector.tensor_mul(ob[:], pacc[:], recip[:])
                        nc.sync.dma_start(O_d[i, pci, :, :], ob[:])
                    else:
                        # ACT evacuates PSUM -> SBUF f32; GpSimd multiplies
                        # by the reciprocal (tail stays off the loaded DVE).
                        ob1 = ep.tile([128, W], F32, tag="ob1")
                        nc.scalar.copy(ob1[:], pacc[:])
                        ob = op.tile([128, W], F32, tag="ob")
                        nc.gpsimd.tensor_mul(ob[:], ob1[:], recip[:])
                        if prouted:
                            # undo the +8 unary shift: out = acc*recip - 8
                            ob2 = op.tile([128, W], F32, tag="ob2")
                            nc.scalar.activation(
                                ob2[:], ob[:],
                                mybir.ActivationFunctionType.Copy,
                                bias=-8.0,
                            )
                            ob = ob2
                        nc.sync.dma_start(O_d[i, pci, :, :], ob[:])

                for ci in [-1] + list(range(NC_CH)):
                    if ci >= CFG["gbuild_at"] and (ci - CFG["gbuild_at"]) % 2 == 0:
                        _emit_gbuild_step(i + 1, n_groups=1)
                    if ci >= 0:
                        for cj in range(ci, min(NC_CH, ci + 1 + CFG["lookahead"])):
                            _ensure_gps(cj)

                    acc = ps.tile([128, W], F32, tag="acc")
                    early = (i == 0 and 0 <= ci < CFG["early_split"]) or (
                        i == CIN - 1 and ci >= NC_CH - CFG["late_split"]
                    )
                    if ci < 0:
                        # norm: accumulate the 24 nonzero g planes directly
                        planes = [(gm, p) for p in range(25) if p != 12]
                    elif (i, ci) in CFG["route"]:
                        # exp-route: arg = -2*d^2 + ln(Q+8) built on PE into
                        # PSUM, one ACT Exp (bias ln wr) emits the fp16
                        # product planes g*(Q+8); the tail's *recip-8 undoes
                        # the +8 shift exactly. Moves the multiply off DVE.
                        lq = lq_tiles[ci]
                        planes = []
                        if CFG["route_group"]:
                            for a in range(5):
                                arg = psa.tile([128, 5, W], F32, tag="arg")
                                nc.tensor.matmul(
                                    arg[:], eye_n2,
                                    _view(d2m, [[1, 5 * W]], elem_offset=5 * a * W),
                                    start=True, stop=False,
                                )
                                nc.tensor.matmul(
                                    arg[:], eye_t[:, 0, :],
                                    _view(lq, [[1, 5], [1, W]], elem_offset=a * WP),
                                    start=False, stop=True,
                                )
                                pbe = ppe.tile([128, 5, W], F16, tag="pbe")
                                nc.scalar.activation(
                                    pbe[:], arg[:],
                                    mybir.ActivationFunctionType.Exp,
                                    bias=bias_t[a][:],
                                )
                                planes += [
                                    (pbe, b) for b in range(5)
                                    if not (a == 2 and b == 2)
                                ]
                        else:
                            for a in range(5):
                                pbe = ppe.tile([128, 5, W], F16, tag="pbe")
                                for b in range(5):
                                    if a == 2 and b == 2:
                                        continue
                                    arg = psa.tile([128, W], F32, tag="arg")
                                    nc.tensor.matmul(
                                        arg[:], eye_n2, d2m[:, 5 * a + b, :],
                                        start=True, stop=False,
                                    )
                                    nc.tensor.matmul(
                                        arg[:], eye_t[:, 0, :],
                                        lq[:, a, b : b + W],
                                        start=False, stop=True,
                                    )
                                    nc.scalar.activation(
                                        pbe[:, b, :], arg[:],
                                        mybir.ActivationFunctionType.Exp,
                                        bias=bias_t[a][:],
                                    )
                                planes += [
                                    (pbe, b) for b in range(5)
                                    if not (a == 2 and b == 2)
                                ]
                        # defer the 24 sum-matmuls so this channel's PSUM
                        # group opens only once the ACT Exps are nearly done
                        deferred[min(ci + CFG["route_defer"], NC_CH - 1)] = (
                            ci,
                            planes,
                            None,
                            True,
                        )
                        continue
                    elif (i, ci) in CFG["route2"]:
                        # partial route: DVE keeps the middle 3 a-groups;
                        # the end groups go through PE+ACT with the +8 shift
                        # cancelled by one -8*partial-norm matmul (sn).
                        lq = lq_tiles[ci]
                        pbv = pp.tile([128, 15, W], F16, tag="pbv")
                        nc.vector.tensor_mul(
                            pbv[:], gm[:, 5:20, :], qwin(ci, 1, 3)
                        )
                        planes = [
                            (pbv, p - 5) for p in range(5, 20) if p != 12
                        ]
                        for a in (0, 4):
                            pbe = ppe.tile([128, 5, W], F16, tag="pbe")
                            for b in range(5):
                                arg = psa.tile([128, W], F32, tag="arg")
                                nc.tensor.matmul(
                                    arg[:], eye_n2, d2m[:, 5 * a + b, :],
                                    start=True, stop=False,
                                )
                                nc.tensor.matmul(
                                    arg[:], eye_t[:, 0, :],
                                    lq[:, a, b : b + W],
                                    start=False, stop=True,
                                )
                                nc.scalar.activation(
                                    pbe[:, b, :], arg[:],
                                    mybir.ActivationFunctionType.Exp,
                                    bias=bias_t[a][:],
                                )
                            planes += [(pbe, b) for b in range(5)]
                        deferred[min(ci + CFG["route_defer"], NC_CH - 1)] = (
                            ci,
                            planes,
                            sn_tiles[i],
                            False,
                        )
                        continue
                    else:
                        dons = gps_emitted[ci]
                        dset = {a for a, _ in dons}

                        if early:
                            # startup: per-a-group products so the first DVE
                            # multiplies start as soon as each Exp lands
                            # instead of waiting for the full g-build chain.
                            pbv = pp.tile([128, 25, W], F16, tag="pbv")
                            for a in range(5):
                                if a in dset:
                                    continue
                                nc.vector.tensor_mul(
                                    pbv[:, 5 * a : 5 * a + 5, :],
                                    gm[:, 5 * a : 5 * a + 5, :],
                                    qwin(ci, a, 1),
                                )
                            planes = [
                                (pbv, p)
                                for p in range(25)
                                if p != 12 and (p // 5) not in dset
                            ]
                        elif ci in pair_tiles:
                            pbv2, lo = pair_tiles.pop(ci)
                            planes = [
                                (pbv2, 20 + p - lo)
                                for p in range(lo, lo + 20)
                                if p != 12
                            ]
                        elif pair_head.get(ci) is not None:
                            # fused product for this donor and its pair two
                            # channels ahead (same donated group): one DVE
                            # instruction, halving the per-instr init cost
                            cj, a_d = pair_head[ci]
                            lo = 5 if a_d == 0 else 0
                            pbv2 = pp.tile([128, 40, W], F16, tag="pbv2")
                            nc.vector.tensor_mul(
                                pbv2[:],
                                _view(gm, [[0, 2], [1, 20 * W]], elem_offset=lo * W),
                                _view(
                                    qm,
                                    [[2 * WP, 2], [QSTRIDE, 4], [1, 5], [1, W]],
                                    elem_offset=(lo // 5) * QSTRIDE + ci * WP,
                                ),
                            )
                            pair_tiles[cj] = (pbv2, lo)
                            planes = [
                                (pbv2, p - lo)
                                for p in range(lo, lo + 20)
                                if p != 12
                            ]
                        else:
                            # DVE covers the contiguous non-donated plane span
                            lo = 5 if 0 in dset else 0
                            hi = 20 if 4 in dset else 25
                            pbv = pp.tile([128, hi - lo, W], F16, tag="pbv")
                            nc.vector.tensor_mul(
                                pbv[:],
                                gm[:, lo:hi, :],
                                qwin(ci, lo // 5, (hi - lo) // 5),
                            )
                            planes = [
                                (pbv, p - lo)
                                for p in range(lo, hi)
                                if p != 12
                            ]
                        for a_d, pbg in dons:
                            planes += [(pbg, p) for p in range(5)]

                    final = (
                        CFG["final_split"]
                        and i == CIN - 1
                        and ci == NC_CH - 1
                    )
                    if final:
                        # last channel: halve the PSUM group by w so the
                        # left half's tail+store overlaps the right half's
                        # matmuls, compressing the end-of-program drain.
                        n_mm = len(planes)
                        hw_ = W // 2
                        for h, x0 in ((0, 0), (1, hw_)):
                            acc_h = ps.tile([128, hw_], F32, tag="acch")
                            for k, (src, p) in enumerate(planes):
                                nc.tensor.matmul(
                                    acc_h[:],
                                    eye_b[p % 5],
                                    src[:, p, x0 : x0 + hw_],
                                    start=(k == 0),
                                    stop=(k == n_mm - 1),
                                )
                                if h == 0 and k == 9:
                                    _flush_tail()
                            ob = op.tile([128, hw_], F32, tag="obh")
                            nc.vector.tensor_mul(
                                ob[:], acc_h[:], recip[:, x0 : x0 + hw_]
                            )
                            nc.sync.dma_start(
                                O_d[i, ci, :, x0 : x0 + hw_], ob[:]
                            )
                        continue
                    n_mm = len(planes)
                    for k, (src, p) in enumerate(planes):
                        # plane index within any a-group aligns mod 5 with the
                        # column offset b (group starts are multiples of 5)
                        b = p % 5
                        nc.tensor.matmul(
                            acc[:],
                            eye_b[b],
                            src[:, p, :],
                            start=(k == 0),
                            stop=(k == n_mm - 1),
                        )
                        if k in (9, 19):
                            _flush_tail()
                    pendings.append((ci, acc, False))
                    if ci in deferred:
                        rci, rplanes, rsn, rflag = deferred.pop(ci)
                        racc = ps.tile([128, W], F32, tag="acc")
                        n_mm = len(rplanes) + (0 if rsn is None else 1)
                        for k, (src, p) in enumerate(rplanes):
                            nc.tensor.matmul(
                                racc[:],
                                eye_b[p % 5],
                                src[:, p, :],
                                start=(k == 0),
                                stop=(k == n_mm - 1),
                            )
                            if k == 9:
                                _flush_tail()
                        if rsn is not None:
                            nc.tensor.matmul(
                                racc[:], eye_n8, rsn[:],
                                start=False, stop=True,
                            )
                        pendings.append((rci, racc, rflag))
                while pendings:
                    _flush_tail()

    nc.compile()
    _PROG_CACHE["nc"] = nc
    return nc


def _make_in_maps(Q, I):
    Q = np.asarray(Q, dtype=np.float32)
    I = np.asarray(I, dtype=np.float32)
    Ip = np.zeros((B, CIN, H + 2 * PAD, WP), np.float16)
    Ip[:, :, PAD : PAD + H, PAD : PAD + W] = I.astype(np.float16)
    Qp = np.zeros((B, NC_CH, H + 2 * PAD, WP), np.float16)
    Qp[:, :, PAD : PAD + H, PAD : PAD + W] = Q.astype(np.float16)
    eye = np.eye(128, dtype=np.float16)
    eye5 = np.ascontiguousarray(
        np.stack(
            [eye, eye * np.float16(_WC[1]), eye * np.float16(_WC[0]),
             eye * np.float16(-2.0), eye * np.float16(-8.0)]
        )
    )
    Lq = np.log(Qp.astype(np.float32) + 8.0).astype(np.float16)
    in_maps = []
    for core in range(N_CORES):
        b, half = divmod(core, 2)
        h0 = half * HOUT
        in_maps.append(
            {
                "I_in": np.ascontiguousarray(Ip[b, :, h0 : h0 + HIN, :]),
                "Q_in": np.ascontiguousarray(Qp[b, :, h0 : h0 + HIN, :]),
                "LQ_in": np.ascontiguousarray(Lq[b, :, h0 : h0 + HIN, :]),
                "EYE5": eye5,
            }
        )
    return in_maps


def _make_d2(Ipf, h0):
    """fp16 d^2 planes [CIN, 128, 25, W] for the shard starting at padded
    row h0 (output rows h0..h0+127)."""
    out = np.empty((CIN, 128, 25, W), np.float16)
    ctr = Ipf[:, h0 + 2 : h0 + 130, 2 : 2 + W]
    for a in range(5):
        for b2 in range(5):
            sh = Ipf[:, h0 + a : h0 + a + 128, b2 : b2 + W]
            d = sh - ctr
            out[:, :, 5 * a + b2, :] = (d * d).astype(np.float16)
    return out


def _assemble(results):
    out = np.zeros((B, CIN, NC_CH, H, W), np.float32)
    for core in range(N_CORES):
        b, half = divmod(core, 2)
        h0 = half * HOUT
        out[b, :, :, h0 : h0 + HOUT, :] = results[core]["OUT"]
    return out


def kernel(Q: np.ndarray, I: np.ndarray) -> np.ndarray:
    nc = _build_program()
    in_maps = _make_in_maps(Q, I)
    res = run_bass_kernel_spmd(nc, in_maps, core_ids=list(range(N_CORES)))
    return _assemble(res.results)
